# revision 3
# baseline (speedup 1.0000x reference)
"""VQ-codebook linear layer on 8 Trainium2 NeuronCores.

Problem: out = x_fp16 @ W_q.T where W_q = codebook[indices].reshape(4096, 4096)
  x:        (4, 2048, 4096) fp32
  codebook: (256, 8) fp16
  indices:  (2097152,) int64
  out:      (4, 2048, 4096) fp32

Sharding: tensor-parallel along out_features (column parallel).
Each core computes out[:, c*512:(c+1)*512] = x @ W[c*512:(c+1)*512, :].T
  -> per-core matmul [8192, 4096] x [4096, 512], fp16 operands, fp32 accum.

Device kernel (per core):
  - weight shard resident in SBUF as W.T tiles [128 k, 512 o], DMAed on the
    ACT (scalar) HWDGE ring so the SP ring starts streaming x immediately
  - stream x.T tiles [128 k, 512 m] from HBM on the SP ring; 4 PSUM banks
    accumulate [128 m, 512 o] tiles over the 32 k-chunks while the other
    4 banks drain, so the PE never stalls at m-group boundaries
  - PSUM -> SBUF drain split across DVE and ACT, output DMA on ACT ring
"""

import numpy as np

import concourse.bacc as bacc
import concourse.mybir as mybir
import concourse.tile as tile
from concourse import bass_utils

B, S, K = 4, 2048, 4096          # batch, seq, in_features
M = B * S                        # 8192 tokens
O = 4096                         # out_features
CORES = 8
OS = O // CORES                  # 512 out_features per core
KC = K // 128                    # 32 k-chunks
MT = 4                           # psum banks (m-tiles of 128) per m-group

_cached = {}


def _build(repeat=1, mt=MT, xt_bufs=6):
    """Build the per-core program. `repeat` emits the whole compute body
    multiple times inside one NEFF (for benchmarking: tunnel-dispatch
    overhead amortizes across repeats; output is written identically each
    repeat so results are unchanged)."""
    mwidth = mt * 128
    mg_count = M // mwidth
    nc = bacc.Bacc("TRN2", target_bir_lowering=False, debug=False)

    xT_d = nc.dram_tensor("xT", [K, M], mybir.dt.float16, kind="ExternalInput")
    wT_d = nc.dram_tensor("wT", [K, OS], mybir.dt.float16, kind="ExternalInput")
    out_d = nc.dram_tensor("out", [M, OS], mybir.dt.float32, kind="ExternalOutput")

    drain = ("vector", "scalar")

    def eng(name):
        return {"sync": nc.sync, "scalar": nc.scalar, "vector": nc.vector}[name]

    def copy_on(name, out, in_):
        e = eng(name)
        if name == "scalar":
            e.copy(out, in_)
        else:
            e.tensor_copy(out=out, in_=in_)

    with tile.TileContext(nc) as tc:
        with (
            tc.tile_pool(name="wt", bufs=1) as wt_pool,
            tc.tile_pool(name="xt", bufs=xt_bufs) as xt_pool,
            tc.tile_pool(name="ot", bufs=8) as out_pool,
            tc.tile_pool(name="ps", bufs=8, space="PSUM") as psum_pool,
        ):
            # resident weight shard: 32 tiles [128, 512] fp16 (4 MB), on the
            # ACT ring so the SP ring's x stream starts in parallel
            wt_tiles = []
            for kc in range(KC):
                wt = wt_pool.tile([128, OS], mybir.dt.float16, tag=f"wt{kc}")
                nc.scalar.dma_start(out=wt[:], in_=wT_d[kc * 128:(kc + 1) * 128, :])
                wt_tiles.append(wt)

            for rep, mg in ((r, g) for r in range(repeat) for g in range(mg_count)):
                m0 = mg * mwidth
                psums = [
                    psum_pool.tile(
                        [128, OS], mybir.dt.float32, tag="ps",
                        name=f"ps{rep}_{mg}_{i}",
                    )
                    for i in range(mt)
                ]
                for kc in range(KC):
                    xt = xt_pool.tile([128, mwidth], mybir.dt.float16, tag="xt")
                    nc.sync.dma_start(
                        out=xt[:],
                        in_=xT_d[kc * 128:(kc + 1) * 128, m0:m0 + mwidth],
                    )
                    for mi in range(mt):
                        nc.tensor.matmul(
                            psums[mi][:],
                            lhsT=xt[:, mi * 128:(mi + 1) * 128],
                            rhs=wt_tiles[kc][:],
                            start=(kc == 0),
                            stop=(kc == KC - 1),
                        )
                for mi in range(mt):
                    o_sb = out_pool.tile(
                        [128, OS], mybir.dt.float32, tag="ot",
                        name=f"ot{rep}_{mg}_{mi}",
                    )
                    copy_on(drain[mi % len(drain)], o_sb[:], psums[mi][:])
                    # alternate output DMAs across both HWDGE rings so the
                    # final group's writes drain in parallel
                    out_eng = nc.scalar if mi % 2 == 0 else nc.sync
                    out_eng.dma_start(
                        out=out_d[m0 + mi * 128:m0 + (mi + 1) * 128, :],
                        in_=o_sb[:],
                    )

    nc.compile()
    return nc


def _prep_inputs(x, codebook, indices):
    codebook = np.asarray(codebook).astype(np.float16, copy=False)
    indices = np.asarray(indices)
    x2 = np.asarray(x).reshape(M, K).astype(np.float16)
    xT = np.ascontiguousarray(x2.T)                       # [K, M] fp16
    W = codebook[indices.astype(np.int64)].reshape(O, K)  # fp16 [4096, 4096]
    in_maps = []
    for c in range(CORES):
        wTc = np.ascontiguousarray(W[c * OS:(c + 1) * OS, :].T)  # [K, OS]
        in_maps.append({"xT": xT, "wT": wTc})
    return in_maps


def kernel(x, codebook, indices):
    if 1 not in _cached:
        _cached[1] = _build(repeat=1)
    nc = _cached[1]
    in_maps = _prep_inputs(x, codebook, indices)
    res = bass_utils.run_bass_kernel_spmd(nc, in_maps, core_ids=list(range(CORES)))
    out = np.concatenate([res.results[c]["out"] for c in range(CORES)], axis=1)
    return out.reshape(B, S, O).astype(np.float32, copy=False)
